# revision 1
# baseline (speedup 1.0000x reference)
"""AudioDecoder (2-layer LSTM, teacher forcing) Bass/Tile kernel for TRN2.

Self-contained: builds the single-core Bass module, shards the batch over 8
NeuronCores (data-parallel, weights replicated), runs via
run_bass_kernel_spmd, and reassembles the full [4096, 100, 80] output.

Sharding: data-parallel over batch (4096 -> 8 cores x 512). Weights replicated.
Layout inside the kernel is feature-major ("transposed"): a [D, B] tensor with
D = n*128 features is stored as an SBUF tile [128, n*512] with feature chunk j
(features j*128+p) at free range [j*512, (j+1)*512).
"""

import numpy as np
from contextlib import ExitStack

import concourse.bass as bass
import concourse.tile as tile
from concourse import bacc
from concourse import mybir
from concourse.masks import make_identity

F32 = mybir.dt.float32
F32R = mybir.dt.float32r
AF = mybir.ActivationFunctionType

LATENT = 128
FEAT = 80
HID = 256
G = 4 * HID  # 1024
BL = 512     # per-core batch
NMB = BL // 128  # 4 batch chunks of 128
MAXT = 100


def r(ap):
    return ap.bitcast(F32R)


def build_kernel(T=MAXT, TC=10, TF=4):
    """Build the single-core Bass module. TC = input DMA chunk (time steps),
    TF = output flush interval (time steps)."""
    assert T <= MAXT and (T - 1) >= 0
    nc = bacc.Bacc()

    z = nc.dram_tensor("z", [BL, LATENT], F32, kind="ExternalInput")
    tseq = nc.dram_tensor("target_seq", [BL, MAXT, FEAT], F32, kind="ExternalInput")
    W_li = nc.dram_tensor("W_li", [FEAT, LATENT], F32, kind="ExternalInput")
    b_li = nc.dram_tensor("b_li", [FEAT], F32, kind="ExternalInput")
    W_ih0 = nc.dram_tensor("W_ih0", [G, FEAT], F32, kind="ExternalInput")
    W_hh0 = nc.dram_tensor("W_hh0", [G, HID], F32, kind="ExternalInput")
    b_ih0 = nc.dram_tensor("b_ih0", [G], F32, kind="ExternalInput")
    b_hh0 = nc.dram_tensor("b_hh0", [G], F32, kind="ExternalInput")
    W_ih1 = nc.dram_tensor("W_ih1", [G, HID], F32, kind="ExternalInput")
    W_hh1 = nc.dram_tensor("W_hh1", [G, HID], F32, kind="ExternalInput")
    b_ih1 = nc.dram_tensor("b_ih1", [G], F32, kind="ExternalInput")
    b_hh1 = nc.dram_tensor("b_hh1", [G], F32, kind="ExternalInput")
    W_fc = nc.dram_tensor("W_fc", [FEAT, HID], F32, kind="ExternalInput")
    b_fc = nc.dram_tensor("b_fc", [FEAT], F32, kind="ExternalInput")
    out = nc.dram_tensor("out", [BL, MAXT, FEAT], F32, kind="ExternalOutput")

    with TileKernel(nc, T, TC, TF) as k:
        k.run(z, tseq, W_li, b_li, W_ih0, W_hh0, b_ih0, b_hh0,
              W_ih1, W_hh1, b_ih1, b_hh1, W_fc, b_fc, out)
    nc.compile()
    return nc


class TileKernel:
    def __init__(self, nc, T, TC, TF):
        self.nc = nc
        self.T, self.TC, self.TF = T, TC, TF
        self.ctx = ExitStack()

    def __enter__(self):
        self.tc = self.ctx.enter_context(tile.TileContext(self.nc))
        return self

    def __exit__(self, *a):
        return self.ctx.__exit__(*a)

    def run(self, z, tseq, W_li, b_li, W_ih0, W_hh0, b_ih0, b_hh0,
            W_ih1, W_hh1, b_ih1, b_hh1, W_fc, b_fc, out):
        nc, tc, ctx = self.nc, self.tc, self.ctx
        T, TC, TF = self.T, self.TC, self.TF

        const = ctx.enter_context(tc.tile_pool(name="const", bufs=1))

        ident0 = const.tile([128, 128], F32, tag="ident0")
        make_identity(nc, ident0)
        ident = const.tile([128, 128], F32R, tag="ident")
        nc.scalar.activation(ident[:], ident0[:], AF.Copy)

        # ---------------- weight prep (transpose to lhsT layouts) -----------
        # WT_ih0 [80, 8*128]: chunk j = W_ih0[j*128:(j+1)*128, :].T
        # WT_hh0 [128, 16*128]: chunk (kk, j) at (kk*8+j)*128
        # WT_ih1 / WT_hh1: same as hh0
        # WfcT_mov [128, 2*80]: chunk kk = W_fc[:, kk*128:(kk+1)*128].T
        # WliT [128, 80] = W_li.T
        wt_ih0 = const.tile([FEAT, 8 * 128], F32R, tag="wt_ih0")
        wt_hh0 = const.tile([128, 16 * 128], F32R, tag="wt_hh0")
        wt_ih1 = const.tile([128, 16 * 128], F32R, tag="wt_ih1")
        wt_hh1 = const.tile([128, 16 * 128], F32R, tag="wt_hh1")
        wfc_mov = const.tile([128, 2 * FEAT], F32R, tag="wfc_mov")
        wli_t = const.tile([128, FEAT], F32R, tag="wli_t")
        b0_sb = const.tile([128, 8], F32, tag="b0_sb")
        b1_sb = const.tile([128, 8], F32, tag="b1_sb")
        bli_sb = const.tile([FEAT, 1], F32, tag="bli_sb")
        bfc4 = const.tile([128, NMB * FEAT], F32, tag="bfc4")
        zt_sb = const.tile([128, BL], F32R, tag="zt_sb")

        with tc.tile_pool(name="setup", bufs=2) as setup, \
             tc.tile_pool(name="setup_ps", bufs=4, space="PSUM") as sps:

            def transpose_to(dst_ap, src_ap):
                # dst[f, p] = src[p, f]; regular fp32r matmul src.T @ I.
                # fp32r needs an even innermost count, so widen N=1 to 2.
                p, fr = src_ap.shape
                n2 = 2 if p == 1 else p
                pst = sps.tile([128, 128], F32, tag="tps")
                nc.tensor.matmul(pst[:fr, :n2], src_ap, ident[:p, :n2],
                                 start=True, stop=True)
                nc.scalar.activation(dst_ap, pst[:fr, :p], AF.Copy)

            # W_ih0 [1024, 80]
            wn = setup.tile([128, 8, FEAT], F32R, tag="wn_ih0")
            nc.sync.dma_start(wn[:], W_ih0[:, :].rearrange("(j p) f -> p j f", p=128).bitcast(F32R))
            for j in range(8):
                transpose_to(wt_ih0[:, j * 128:(j + 1) * 128], wn[:, j, :])

            # W_hh0 / W_ih1 / W_hh1 [1024, 256]
            for W, wt, tg in ((W_hh0, wt_hh0, "wn_hh0"), (W_ih1, wt_ih1, "wn_ih1"),
                              (W_hh1, wt_hh1, "wn_hh1")):
                wn = setup.tile([128, 8, HID], F32R, tag=tg)
                nc.sync.dma_start(wn[:], W[:, :].rearrange("(j p) f -> p j f", p=128).bitcast(F32R))
                for j in range(8):
                    for kk in range(2):
                        transpose_to(wt[:, (kk * 8 + j) * 128:(kk * 8 + j + 1) * 128],
                                     wn[:, j, kk * 128:(kk + 1) * 128])

            # W_fc [80, 256] -> moving rhs [256, 80] chunks
            wn = setup.tile([FEAT, HID], F32R, tag="wn_fc")
            nc.sync.dma_start(wn[:], W_fc[:, :].bitcast(F32R))
            for kk in range(2):
                transpose_to(wfc_mov[:, kk * FEAT:(kk + 1) * FEAT],
                             wn[:, kk * 128:(kk + 1) * 128])

            # W_li [80, 128] -> [128, 80]
            wn = setup.tile([FEAT, LATENT], F32R, tag="wn_li")
            nc.sync.dma_start(wn[:], W_li[:, :].bitcast(F32R))
            transpose_to(wli_t[:, :], wn[:, :])

            # biases: b0 = b_ih0 + b_hh0 folded to [128, 8]; same for b1.
            for ba, bb, bsb, tg in ((b_ih0, b_hh0, b0_sb, "b0"),
                                    (b_ih1, b_hh1, b1_sb, "b1")):
                ta = setup.tile([1, G], F32, tag=tg + "a")
                tb = setup.tile([1, G], F32, tag=tg + "b")
                nc.sync.dma_start(ta[:], ba[None, :])
                nc.sync.dma_start(tb[:], bb[None, :])
                tsum = setup.tile([1, G], F32R, tag=tg + "s")
                nc.vector.tensor_add(tsum[:], ta[:], tb[:])
                for j in range(8):
                    transpose_to(bsb[:, j:j + 1], tsum[:, j * 128:(j + 1) * 128])

            tb = setup.tile([1, FEAT], F32R, tag="bli")
            nc.sync.dma_start(tb[:], b_li[None, :].bitcast(F32R))
            transpose_to(bli_sb[:, :], tb[:, :])

            # b_fc broadcast to [128, 4*80]
            bfc_ap = b_fc[:]
            bfc_b = bass.AP(tensor=bfc_ap.tensor, offset=bfc_ap.offset,
                            ap=[[0, 128], [1, FEAT]])
            nc.gpsimd.dma_start(bfc4[:, 0:FEAT], bfc_b)
            for mb in range(1, NMB):
                nc.vector.tensor_copy(bfc4[:, mb * FEAT:(mb + 1) * FEAT],
                                      bfc4[:, 0:FEAT])

            # z.T [128, 512]
            zn = setup.tile([128, NMB, LATENT], F32R, tag="zn")
            nc.sync.dma_start(zn[:], z[:, :].rearrange("(mb p) l -> p mb l", p=128).bitcast(F32R))
            zps = sps.tile([128, BL], F32, tag="zps")
            for mb in range(NMB):
                nc.tensor.matmul(zps[:, mb * 128:(mb + 1) * 128], zn[:, mb, :],
                                 ident[:, :], start=True, stop=True)
            nc.scalar.activation(zt_sb[:], zps[:], AF.Copy)

        # ---------------- pools for the time loop ---------------------------
        pg = ctx.enter_context(tc.tile_pool(name="pg", bufs=4, space="PSUM"))
        pxt = ctx.enter_context(tc.tile_pool(name="pxt", bufs=2, space="PSUM"))
        py = ctx.enter_context(tc.tile_pool(name="py", bufs=1, space="PSUM"))
        acts = ctx.enter_context(tc.tile_pool(name="acts", bufs=2))
        tmp = ctx.enter_context(tc.tile_pool(name="tmp", bufs=2))
        states = ctx.enter_context(tc.tile_pool(name="states", bufs=2))
        xpool = ctx.enter_context(tc.tile_pool(name="xpool", bufs=3))
        xin = ctx.enter_context(tc.tile_pool(name="xin", bufs=3))
        ypool = ctx.enter_context(tc.tile_pool(name="ypool", bufs=2))

        # states start at zero; t=0 skips all h/c-dependent terms instead
        # of memsetting (memset can't write fp32r).
        h0T = c0T = h1T = c1T = None

        # Collapse setup-phase dependency fan-in so loop instructions don't
        # accumulate too many semaphore waits (walrus per-inst wait limit:
        # 4-byte self-loading matmuls can encode only ONE sync wait).
        tc.strict_bb_all_engine_barrier()

        # x0.T = W_li @ z.T + b_li  (into the step-0 xT slot)
        x0ps = pxt.tile([FEAT, BL], F32, tag="xt_ps")
        nc.tensor.matmul(x0ps[:], (wli_t[:]), (zt_sb[:]), start=True, stop=True)
        xT = xpool.tile([FEAT, BL], F32R, tag="xT")
        nc.scalar.activation(xT[:], x0ps[:], AF.Identity, bias=bli_sb[:, 0:1])

        # input chunks: x for step t (1..T-1) is tseq[:, t-1, :]
        n_chunks = (T - 1 + TC - 1) // TC if T > 1 else 0
        xchunks, xdmas = [], []
        for c in range(n_chunks):
            tlo = c * TC
            thi = min((c + 1) * TC, T - 1)
            xc = xin.tile([128, NMB, TC, FEAT], F32R, tag="xc")
            d = nc.sync.dma_start(
                xc[:, :, 0:thi - tlo, :],
                tseq[:, tlo:thi, :].rearrange("(mb p) t f -> p mb t f", p=128).bitcast(F32R))
            xchunks.append(xc)
            xdmas.append(d)

        ystage = None
        GATE_FUNCS = (AF.Sigmoid, AF.Sigmoid, AF.Tanh, AF.Sigmoid)

        for t in range(T):
            # ---- layer 0 gates: G0_j = Wih0_j @ xT + Whh0_j @ h0T (+ b0) ----
            a0 = [acts.tile([128, 2 * BL], F32, tag=f"a0_{i}", name=f"a0_{i}_{t}") for i in range(4)]
            a0_rel = None
            for j in range(8):
                g = pg.tile([128, BL], F32, tag="g")
                nc.tensor.matmul(g[:], (wt_ih0[:, j * 128:(j + 1) * 128]),
                                 (xT[:]), start=True, stop=(t == 0))
                if t > 0:
                    for kk in range(2):
                        nc.tensor.matmul(
                            g[:], (wt_hh0[:, (kk * 8 + j) * 128:(kk * 8 + j + 1) * 128]),
                            (h0T[:, kk * BL:(kk + 1) * BL]),
                            start=False, stop=(kk == 1))
                nc.scalar.activation(
                    a0[j // 2][:, (j % 2) * BL:(j % 2 + 1) * BL],
                    g[:], GATE_FUNCS[j // 2], bias=b0_sb[:, j:j + 1])

            # ---- prep x for step t+1 (off critical path) ----
            if t + 1 < T:
                c_idx, slot = t // TC, t % TC
                xt_ps = pxt.tile([FEAT, BL], F32, tag="xt_ps")
                for mb in range(NMB):
                    nc.tensor.matmul(xt_ps[:, mb * 128:(mb + 1) * 128],
                                     xchunks[c_idx][:, mb, slot, :], ident[:, :],
                                     start=True, stop=True)
                xT_next = xpool.tile([FEAT, BL], F32R, tag="xT")
                nc.vector.tensor_copy(xT_next[:], xt_ps[:])
            else:
                xT_next = None

            # ---- layer 0 elementwise ----
            c0T_n = states.tile([128, 2 * BL], F32, tag="c0T")
            if t == 0:
                nc.vector.tensor_mul(c0T_n[:], a0[0][:], a0[2][:])
            else:
                ig0 = tmp.tile([128, 2 * BL], F32, tag="tmp")
                nc.vector.tensor_mul(ig0[:], a0[0][:], a0[2][:])
                fc0 = tmp.tile([128, 2 * BL], F32, tag="tmp")
                nc.vector.tensor_mul(fc0[:], a0[1][:], c0T[:])
                nc.vector.tensor_add(c0T_n[:], ig0[:], fc0[:])
            tc0 = tmp.tile([128, 2 * BL], F32, tag="tmp")
            nc.scalar.activation(tc0[:], c0T_n[:], AF.Tanh)
            h0T_n = states.tile([128, 2 * BL], F32R, tag="h0T")
            nc.vector.tensor_mul(h0T_n[:], a0[3][:], tc0[:])

            # ---- layer 1 gates: G1_j = Wih1_j @ h0T_n + Whh1_j @ h1T (+ b1) ----
            a1 = [acts.tile([128, 2 * BL], F32, tag=f"a1_{i}", name=f"a1_{i}_{t}") for i in range(4)]
            for j in range(8):
                g = pg.tile([128, BL], F32, tag="g")
                for kk in range(2):
                    nc.tensor.matmul(
                        g[:], (wt_ih1[:, (kk * 8 + j) * 128:(kk * 8 + j + 1) * 128]),
                        (h0T_n[:, kk * BL:(kk + 1) * BL]),
                        start=(kk == 0), stop=(t == 0 and kk == 1))
                if t > 0:
                    for kk in range(2):
                        nc.tensor.matmul(
                            g[:], (wt_hh1[:, (kk * 8 + j) * 128:(kk * 8 + j + 1) * 128]),
                            (h1T[:, kk * BL:(kk + 1) * BL]),
                            start=False, stop=(kk == 1))
                nc.scalar.activation(
                    a1[j // 2][:, (j % 2) * BL:(j % 2 + 1) * BL],
                    g[:], GATE_FUNCS[j // 2], bias=b1_sb[:, j:j + 1])

            # ---- layer 1 elementwise ----
            c1T_n = states.tile([128, 2 * BL], F32, tag="c1T")
            if t == 0:
                nc.vector.tensor_mul(c1T_n[:], a1[0][:], a1[2][:])
            else:
                ig1 = tmp.tile([128, 2 * BL], F32, tag="tmp")
                nc.vector.tensor_mul(ig1[:], a1[0][:], a1[2][:])
                fc1 = tmp.tile([128, 2 * BL], F32, tag="tmp")
                nc.vector.tensor_mul(fc1[:], a1[1][:], c1T[:])
                nc.vector.tensor_add(c1T_n[:], ig1[:], fc1[:])
            tc1 = tmp.tile([128, 2 * BL], F32, tag="tmp")
            nc.scalar.activation(tc1[:], c1T_n[:], AF.Tanh)
            h1T_n = states.tile([128, 2 * BL], F32R, tag="h1T")
            nc.vector.tensor_mul(h1T_n[:], a1[3][:], tc1[:])

            # ---- fc: y[b, f] batch-major; lhsT = h1T_n chunks ----
            if t % TF == 0:
                ystage = ypool.tile([128, NMB, TF, FEAT], F32, tag="ystage")
            yps = py.tile([128, NMB * FEAT], F32, tag="yps")
            for mb in range(NMB):
                for kk in range(2):
                    nc.tensor.matmul(
                        yps[:, mb * FEAT:(mb + 1) * FEAT],
                        (h1T_n[:, kk * BL + mb * 128:kk * BL + (mb + 1) * 128]),
                        (wfc_mov[:, kk * FEAT:(kk + 1) * FEAT]),
                        start=(kk == 0), stop=(kk == 1))
            nc.vector.tensor_add(
                ystage[:, :, t % TF, :],
                yps[:].rearrange("p (mb f) -> p mb f", mb=NMB),
                bfc4[:].rearrange("p (mb f) -> p mb f", mb=NMB))

            # ---- flush output every TF steps ----
            if t % TF == TF - 1 or t == T - 1:
                t0 = (t // TF) * TF
                nf = t - t0 + 1
                out_r = out[:, :, :].rearrange("(mb p) t f -> mb p t f", p=128)
                for mb in range(NMB):
                    nc.sync.dma_start(out_r[mb, :, t0:t0 + nf, :],
                                      ystage[:, mb, 0:nf, :])

            h0T, c0T, h1T, c1T = h0T_n, c0T_n, h1T_n, c1T_n
            xT = xT_next




B_FULL = 4096
N_CORES = 8

_nc_cache = {}


def _get_nc(T=MAXT):
    if T not in _nc_cache:
        _nc_cache[T] = build_kernel(T=T)
    return _nc_cache[T]


def make_in_maps(inputs, n_cores=N_CORES):
    """Split full inputs into per-core maps (batch sharded, weights replicated)."""
    inp = {k: np.ascontiguousarray(np.asarray(v, dtype=np.float32))
           for k, v in inputs.items()}
    assert inp["z"].shape == (B_FULL, LATENT)
    maps = []
    for c in range(n_cores):
        sl = slice(c * BL, (c + 1) * BL)
        m = dict(inp)
        m["z"] = inp["z"][sl]
        m["target_seq"] = np.ascontiguousarray(inp["target_seq"][sl])
        maps.append(m)
    return maps


def kernel(**inputs) -> np.ndarray:
    from concourse.bass_utils import run_bass_kernel_spmd

    nc = _get_nc()
    maps = make_in_maps(inputs)
    res = run_bass_kernel_spmd(nc, maps, core_ids=list(range(N_CORES)))
    return np.concatenate([res.results[c]["out"] for c in range(N_CORES)], axis=0)
